# revision 7
# baseline (speedup 1.0000x reference)
"""Multi-head attention (B=4, T=2048, D=2048, H=16, E=128) on 8 TRN2 NeuronCores.

v11 = v9 + DMA prefetch of weights one head ahead:
  - wq/wk for head h+1 are DMA-issued BEFORE head h's attention is emitted,
    so they sit ahead of h's output DMAs in the SP queue and land before
    head h+1's projections need them (the last out-DMA gates the queue on
    the final normalize-multiply otherwise).
  - wv for quad q+1 issued at the start of head 4q+2's attention; quad 0's
    wv issued right after the startup triplets. (wvp bufs 1 -> 2.)
  PE instruction order is unchanged.

v9 = v7 + deferred denominator tail only (ct stays 1 behind):
  - the per-qc denominator matmul (ones @ d16) and its recip/mul/out-DMA
    tail are emitted 4 score-MMs INTO THE NEXT q-chunk, so the PE's FIFO
    head never waits on the exp15 -> DVE-tree chain (~2-3us) with only one
    context matmul of cover; the next qc's score MMs fill the gap.
  Pure reordering: numerics are bit-identical.

v7 = baseline + chunk-major startup: head-0's Q/K projection matmuls are
emitted chunk-major across 7 concurrent PSUM chains with per-chunk
interleaved weight/x DMAs, so the PE consumes each arriving x chunk at
~the DMA cadence instead of idling ~1.2us per chunk while chain 0 waits
for the full 8MB x stream. Steady state and numerics are identical.

Sharding: batch (4) x head-group (2 groups of 8 heads) -> 8 cores.
Per core: q/k/v projections for its 8 heads + softmax(QK^T/sqrt(E))V.

Layout strategy (no on-chip transposes are ever needed):
  - host passes x^T [D,T] and per-head W^T [D,E] (bf16) so the contraction
    dim D lands on SBUF partitions directly.
  - Q^T,K^T computed as [E,T] (lhsT=W^T chunk, rhs=x^T chunk).
  - V computed as [T,E] (lhsT=x^T chunk, rhs=Wv^T chunk), head-quads at N=512.
  - scores computed transposed: S^T[k,q] = (K Q^T), softmax-exp on ACT,
    P^T feeds C^T[e,q] = V^T P^T; softmax denominators: DVE tree-sums all
    16 P^T tiles of a q-chunk elementwise (bf16, depth-4 tree), then a
    single all-ones matmul per q-chunk reduces over partitions into PSUM
    (15/16 of the denominator's PE streaming moved to the idle DVE).
  - output written as C^T [h,E,T]; host transposes back to [h,T,E].
"""

import math
import sys

sys.path.insert(0, "/opt/trn_rl_repo")

import ml_dtypes
import numpy as np

import concourse.bass as bass  # noqa: F401  (registers engine methods)
import concourse.mybir as mybir
import concourse.tile as tile
from concourse import bacc
from concourse.bass_utils import run_bass_kernel_spmd

B, T, D, H, E = 4, 2048, 2048, 16, 128
N_CORES = 8
H_LOC = H // 2          # heads per core
P = 128                 # partitions
DT = D // P             # contraction chunks for projections
KT = T // P             # key tiles
QW = 512                # q-chunk width (one PSUM bank of fp32)
QC = T // QW
BF16 = mybir.dt.bfloat16
F32 = mybir.dt.float32
EXP_SCALE = 1.0 / math.sqrt(E)


def _build(repeat=1, io_internal=False, barrier=False):
    nc = bacc.Bacc("TRN2", target_bir_lowering=False, debug=False,
                   num_devices=N_CORES)
    ik = "Internal" if io_internal else "ExternalInput"
    ok = "Internal" if io_internal else "ExternalOutput"
    xT = nc.dram_tensor("xT", [D, T], BF16, kind=ik).ap()
    wqT = nc.dram_tensor("wqT", [H_LOC, D, E], BF16, kind=ik).ap()
    wkT = nc.dram_tensor("wkT", [H_LOC, D, E], BF16, kind=ik).ap()
    wvT = nc.dram_tensor("wvT", [H_LOC // 4, D, 4 * E], BF16,
                         kind=ik).ap()
    out = nc.dram_tensor("out", [H_LOC, E, T], F32, kind=ok).ap()
    dummy = (nc.dram_tensor("dummy_out", [1, 4], F32, kind="ExternalOutput").ap()
             if io_internal else None)

    with tile.TileContext(nc) as tc:
        with (
            tc.tile_pool(name="xpool", bufs=1) as xpool,
            tc.tile_pool(name="wqk", bufs=2) as wqk,
            tc.tile_pool(name="wvp", bufs=2) as wvp,
            tc.tile_pool(name="qk", bufs=2) as qk,
            tc.tile_pool(name="vpool", bufs=2) as vpool,
            tc.tile_pool(name="ptp", bufs=8) as ptp,
            tc.tile_pool(name="outp", bufs=4) as outp,
            tc.tile_pool(name="smallp", bufs=3) as smallp,
            tc.tile_pool(name="dsum", bufs=3) as dsum,
            tc.tile_pool(name="onesp", bufs=1) as onesp,
            tc.tile_pool(name="stps", bufs=3, space="PSUM") as stps,
            tc.tile_pool(name="projps", bufs=2, space="PSUM") as projps,
            tc.tile_pool(name="ctps", bufs=2, space="PSUM") as ctps,
            tc.tile_pool(name="sumps", bufs=1, space="PSUM") as sumps,
        ):
            for _rep in range(repeat):
                _kernel_rep(tc, nc, xpool, wqk, wvp, qk, vpool, ptp, outp,
                            smallp, onesp, stps, projps, ctps, sumps,
                            xT, wqT, wkT, wvT, out, dsum, barrier=barrier)
            if dummy is not None:
                dt_sb = smallp.tile([1, 4], F32, tag="dummy")
                nc.vector.memset(dt_sb[:], 1.0)
                nc.sync.dma_start(dummy, dt_sb[:])
    nc.compile()
    return nc


def _kernel_rep(tc, nc, xpool, wqk, wvp, qk, vpool, ptp, outp, smallp, onesp,
                stps, projps, ctps, sumps, xT, wqT, wkT, wvT, out, dsum,
                barrier=False):
    ones = onesp.tile([P, P], BF16)
    nc.vector.memset(ones[:], 1.0)

    def _load_w(h):
        wq_sb = wqk.tile([P, DT, E], BF16, tag="wq")
        nc.sync.dma_start(wq_sb[:], wqT[h].rearrange("(c p) e -> p c e", p=P))
        wk_sb = wqk.tile([P, DT, E], BF16, tag="wk")
        nc.sync.dma_start(wk_sb[:], wkT[h].rearrange("(c p) e -> p c e", p=P))
        return wq_sb, wk_sb

    # Startup DMA order: per-chunk (wq0_c, wk0_c, x_c) triplets so the
    # chunk-major head-0 projection can start after ~one chunk's worth of
    # DMA instead of the whole x stream.
    xTr = xT.rearrange("(c p) t -> p c t", p=P)
    wq0r = wqT[0].rearrange("(c p) e -> p c e", p=P)
    wk0r = wkT[0].rearrange("(c p) e -> p c e", p=P)
    xs = []
    for c in range(DT):
        xt = xpool.tile([P, T], BF16, tag=f"x{c}")
        xs.append(xt)
    wq0_sb = wqk.tile([P, DT, E], BF16, tag="wq")
    wk0_sb = wqk.tile([P, DT, E], BF16, tag="wk")
    for c in range(DT):
        nc.sync.dma_start(wq0_sb[:, c:c + 1, :], wq0r[:, c:c + 1, :])
        nc.sync.dma_start(wk0_sb[:, c:c + 1, :], wk0r[:, c:c + 1, :])
        nc.sync.dma_start(xs[c][:], xTr[:, c, :])
    w0 = (wq0_sb, wk0_sb)

    def _proj_qk(h, w=None, boost=False):
        # ---- Q^T / K^T projections for head h: [E, T] ----
        # boost (head 0 only): run 7 chains chunk-major across 7 PSUM banks
        # (2 proj + 2 ct + 1 sum + 2 st, all idle at startup) so the PE
        # consumes each x chunk as it lands; the 8th chain runs after, by
        # which time x is resident. Per-chain accumulation order (dt 0..15)
        # is unchanged -> numerics identical to the plain path.
        wq_sb, wk_sb = w if w is not None else _load_w(h)
        qT = qk.tile([P, T], BF16, tag="qT")
        kT_sb = qk.tile([P, T], BF16, tag="kT")
        specs = [(wq_sb, qT, nt) for nt in range(QC)] + \
                [(wk_sb, kT_sb, nt) for nt in range(QC)]
        if boost:
            pools7 = [(projps, "proj"), (projps, "proj"), (ctps, "ct"),
                      (ctps, "ct"), (sumps, "sum"), (stps, "st"),
                      (stps, "st")]
            ps7 = []
            for pi, (pool, tag) in enumerate(pools7):
                ps7.append(pool.tile([P, QW], F32, tag=tag,
                                     name=f"ps7_{pi}"))
            for dt_i in range(DT):
                for ci in range(7):
                    w_sb, oT, nt = specs[ci]
                    nc.tensor.matmul(
                        ps7[ci][:], lhsT=w_sb[:, dt_i, :],
                        rhs=xs[dt_i][:, nt * QW:(nt + 1) * QW],
                        start=(dt_i == 0), stop=(dt_i == DT - 1))
            for ci in range(7):
                w_sb, oT, nt = specs[ci]
                nc.vector.tensor_copy(oT[:, nt * QW:(nt + 1) * QW],
                                      ps7[ci][:])
            rest = specs[7:]
        else:
            rest = specs
        for w_sb, oT, nt in rest:
            ps = projps.tile([P, QW], F32, tag="proj")
            for dt_i in range(DT):
                nc.tensor.matmul(
                    ps[:], lhsT=w_sb[:, dt_i, :],
                    rhs=xs[dt_i][:, nt * QW:(nt + 1) * QW],
                    start=(dt_i == 0), stop=(dt_i == DT - 1))
            nc.vector.tensor_copy(oT[:, nt * QW:(nt + 1) * QW], ps[:])
        return qT, kT_sb

    def _attn(h, hi, qT, kT_sb, v_sb):
        # ---- attention for head h (ct sw-pipelined TWO kt behind; the
        # denominator matmul + output tail of qc are deferred into qc+1) ----
        stash = [None]
        for qc in range(QC):
            ct = ctps.tile([P, QW], F32, tag="ct")
            pts = [None] * KT
            d12_prev = [None]
            d8_prev = [None]

            def _ct(kt, ct=ct, pts=pts):
                nc.tensor.matmul(
                    ct[:], lhsT=v_sb[:, kt, hi * E:(hi + 1) * E],
                    rhs=pts[kt],
                    start=(kt == 0), stop=(kt == KT - 1))

            for kt in range(KT):
                st = stps.tile([P, QW], F32, tag="st")
                nc.tensor.matmul(
                    st[:], lhsT=kT_sb[:, kt * P:(kt + 1) * P],
                    rhs=qT[:, qc * QW:(qc + 1) * QW],
                    start=True, stop=True)
                pt = ptp.tile([P, QW], BF16, tag="pt")
                nc.scalar.activation(
                    pt[:], st[:], mybir.ActivationFunctionType.Exp,
                    scale=EXP_SCALE)
                pts[kt] = pt[:]
                if kt == 4 and stash[0] is not None:
                    stash[0]()
                    stash[0] = None
                if kt >= 1:
                    _ct(kt - 1)
                if kt % 4 == 3:
                    # DVE tree-sums 8 P^T tiles; one denominator matmul per 8
                    d1 = dsum.tile([P, QW], BF16, tag="d1")
                    nc.vector.tensor_add(d1[:], pts[kt - 3], pts[kt - 2])
                    d2 = dsum.tile([P, QW], BF16, tag="d2")
                    nc.vector.tensor_add(d2[:], pts[kt - 1], pts[kt])
                    d12 = dsum.tile([P, QW], BF16, tag="d12")
                    nc.vector.tensor_add(d12[:], d1[:], d2[:])
                    if kt % 8 == 3:
                        d12_prev[0] = d12
                    else:
                        d8 = dsum.tile([P, QW], BF16, tag="d8")
                        nc.vector.tensor_add(d8[:], d12_prev[0][:], d12[:])
                        if kt == 7:
                            d8_prev[0] = d8
                        else:
                            d16 = dsum.tile([P, QW], BF16, tag="d16")
                            nc.vector.tensor_add(d16[:], d8_prev[0][:], d8[:])

                            def tail(qc=qc, ct=ct, d16=d16):
                                sm = sumps.tile([P, QW], F32, tag="sum",
                                                name="sm")
                                nc.tensor.matmul(
                                    sm[:], lhsT=ones[:], rhs=d16[:],
                                    start=True, stop=True)
                                rec = smallp.tile([P, QW], F32, tag="rec",
                                                  name="rec")
                                nc.vector.reciprocal(rec[:], sm[:])
                                ot = outp.tile([P, QW], F32, tag="ot",
                                               name="ot")
                                nc.vector.tensor_mul(ot[:], ct[:], rec[:])
                                nc.sync.dma_start(
                                    out[h, :, qc * QW:(qc + 1) * QW], ot[:])
                            stash[0] = tail
            _ct(KT - 1)
        stash[0]()
        stash[0] = None

    def _load_wv(quad):
        wv_sb = wvp.tile([P, DT, 4 * E], BF16, tag="wv")
        wvr = wvT[quad].rearrange("(c p) e -> p c e", p=P)
        for c4 in range(0, DT, 4):
            nc.sync.dma_start(wv_sb[:, c4:c4 + 4, :], wvr[:, c4:c4 + 4, :])
        return wv_sb

    wv_next = [_load_wv(0)]
    w_next = [w0]
    for quad in range(H_LOC // 4):
        # head-0 Q/K proj first: small weight DMAs -> PE starts early
        qk0 = _proj_qk(4 * quad, w=w_next[0], boost=(quad == 0))

        # ---- V projection for the head quad: v_sb[k, kt, 4E] ----
        wv_sb = wv_next[0]
        v_sb = vpool.tile([P, KT, 4 * E], BF16, tag="v")
        for kt in range(KT):
            ps = projps.tile([P, 4 * E], F32, tag="proj")
            for dt_i in range(DT):
                nc.tensor.matmul(
                    ps[:], lhsT=xs[dt_i][:, kt * P:(kt + 1) * P],
                    rhs=wv_sb[:, dt_i, :],
                    start=(dt_i == 0), stop=(dt_i == DT - 1))
            nc.vector.tensor_copy(v_sb[:, kt, :], ps[:])

        for hi in range(4):
            h = 4 * quad + hi
            qT, kT_sb = (qk0 if hi == 0
                         else _proj_qk(h, w=w_next[0]))
            # prefetch next head's (and next quad's) weights so their DMAs
            # queue ahead of this head's output DMAs
            if h + 1 < H_LOC:
                w_next[0] = _load_w(h + 1)
            if hi == 2 and quad + 1 < H_LOC // 4:
                wv_next[0] = _load_wv(quad + 1)
            _attn(h, hi, qT, kT_sb, v_sb)

    if barrier:
        # timing-only: touch every x tile at rep end so the next rep's x
        # DMAs (WAR) cannot prefetch -> every rep pays true startup cost
        bar = smallp.tile([P, DT], F32, tag="bar")
        for c in range(DT):
            nc.vector.tensor_copy(bar[:, c:c + 1], xs[c][:, 0:1])


_NC_CACHE = {}


def _get_nc():
    if "nc" not in _NC_CACHE:
        _NC_CACHE["nc"] = _build()
    return _NC_CACHE["nc"]


def _prep_in_maps(x, Wq, Wk, Wv):
    bf = ml_dtypes.bfloat16
    x16 = np.asarray(x).astype(bf)
    Wq16 = np.asarray(Wq).astype(bf)
    Wk16 = np.asarray(Wk).astype(bf)
    Wv16 = np.asarray(Wv).astype(bf)

    xT_by_b = [np.ascontiguousarray(x16[b].T) for b in range(B)]
    wq_by_g, wk_by_g, wv_by_g = [], [], []
    for g in range(2):
        sl = slice(g * H_LOC * E, (g + 1) * H_LOC * E)
        wq_by_g.append(np.ascontiguousarray(
            Wq16[sl].reshape(H_LOC, E, D).transpose(0, 2, 1)))
        wk_by_g.append(np.ascontiguousarray(
            Wk16[sl].reshape(H_LOC, E, D).transpose(0, 2, 1)))
        wv_by_g.append(np.ascontiguousarray(
            Wv16[sl].reshape(H_LOC // 4, 4, E, D)
            .transpose(0, 3, 1, 2).reshape(H_LOC // 4, D, 4 * E)))

    in_maps = []
    for c in range(N_CORES):
        b, g = divmod(c, 2)
        in_maps.append({
            "xT": xT_by_b[b],
            "wqT": wq_by_g[g],
            "wkT": wk_by_g[g],
            "wvT": wv_by_g[g],
        })
    return in_maps


def run_sharded(x, Wq, Wk, Wv, **spmd_kwargs):
    """Build+run on 8 cores; returns (full_output, BassKernelResults)."""
    nc = _get_nc()
    in_maps = _prep_in_maps(x, Wq, Wk, Wv)
    res = run_bass_kernel_spmd(nc, in_maps, list(range(N_CORES)), **spmd_kwargs)
    full = np.empty((B, H, T, E), np.float32)
    for c in range(N_CORES):
        b, g = divmod(c, 2)
        oc = res.results[c]["out"]  # [H_LOC, E, T]
        full[b, g * H_LOC:(g + 1) * H_LOC] = oc.transpose(0, 2, 1)
    return full, res


def kernel(x, Wq, Wk, Wv):
    full, _ = run_sharded(x, Wq, Wk, Wv)
    return full

